# revision 1
# baseline (speedup 1.0000x reference)
"""Sigmoid-attention MHA kernel for 8 Trainium2 NeuronCores.

Problem: x[4,2048,512], W_q/W_k/W_v/W_o[512,512] (already scaled).
  Q = x@Wq.T, K = x@Wk.T, V = x@Wv.T split into 8 heads of depth 64
  attn = sigmoid(QK^T/sqrt(64) - log(2048));  out = (attn@V merged)@Wo.T

Sharding: core c handles batch b=c//2, head-group g=c%2 (4 heads each).
Each core computes a partial output projection over its 256 head-features;
host sums the two partials per batch.

On-chip layout (per core):
  xt   [128,8192]  x[b].T chunked   xt[p, 2048*kc+t] = x[b].T[128*kc+p, t]
  qt/kt (2 tiles [128,2048])        Q^T/K^T rows=features, cols=tokens
  v    (16 tiles [128,256])         V natural rows=tokens
  scores.T computed per (head-pair, 128-key-chunk, 512-query-chunk) into
  PSUM [128,1024] (two heads side by side), sigmoid on ScalarE (fused
  0.125 scale + -log(2048) bias) -> SBUF, then attn.T@V accumulated over
  key chunks into per-head PSUM tiles -> O^T tiles -> output projection.
"""

import os
import numpy as np

DEBUG = bool(int(os.environ.get("KERNEL_DEBUG", "0")))
LOOP = int(os.environ.get("KERNEL_LOOP", "0"))  # >0: wrap body in For_i (timing)
ABLATE = os.environ.get("KERNEL_ABLATE", "")  # timing experiments only
PTAIL = bool(int(os.environ.get("KERNEL_PTAIL", "1")))
JITV = bool(int(os.environ.get("KERNEL_JITV", "1")))  # P-proj at tail instead
XPAIR = bool(int(os.environ.get("KERNEL_XPAIR", "0")))  # one pipeline across pairs

B, S, D = 4, 2048, 512
NH, DEPTH = 8, 64
G = 2          # head groups (one per core pair)
GF = 256       # features per group
NEG_LOG_S = float(np.float32(-np.log(np.float32(S))))
INV_SQRT_DK = 0.125

_CACHE = {}


def _build_nc():
    import concourse.bacc as bacc
    import concourse.tile as tile
    from concourse import mybir

    f32 = mybir.dt.float32
    f32r = mybir.dt.float32r
    nc = bacc.Bacc("TRN2", target_bir_lowering=False, debug=False, num_devices=8)

    xt_d = nc.dram_tensor("xt", [128, 8192], f32r, kind="ExternalInput").ap()
    wq_d = nc.dram_tensor("wq", [128, 1024], f32r, kind="ExternalInput").ap()
    wk_d = nc.dram_tensor("wk", [128, 1024], f32r, kind="ExternalInput").ap()
    wv_d = nc.dram_tensor("wv", [128, 1024], f32r, kind="ExternalInput").ap()
    wo_d = nc.dram_tensor("wo", [128, 1024], f32r, kind="ExternalInput").ap()
    out_d = nc.dram_tensor("out", [S, D], f32, kind="ExternalOutput").ap()
    dbg = {}
    if DEBUG:
        for nm in ("qt", "kt"):
            dbg[nm] = [nc.dram_tensor(f"dbg_{nm}{m}", [128, 2048], f32r,
                                      kind="ExternalOutput").ap() for m in range(2)]
        dbg["ot"] = [nc.dram_tensor(f"dbg_ot{m}", [128, 2048], f32r,
                                    kind="ExternalOutput").ap() for m in range(2)]
        dbg["v"] = [nc.dram_tensor(f"dbg_v{t}", [128, 256], f32r,
                                   kind="ExternalOutput").ap() for t in range(16)]

    with tile.TileContext(nc) as tc:
        with (
            tc.tile_pool(name="persist", bufs=1) as persist,
            tc.tile_pool(name="attn", bufs=int(os.environ.get("KERNEL_ABUFS", "6"))) as apool,
            tc.tile_pool(name="stage", bufs=4) as stage,
            tc.tile_pool(name="spsum", bufs=3, space="PSUM") as spsum,
            tc.tile_pool(name="opsum", bufs=1, space="PSUM") as opsum,
        ):
            import contextlib
            LOOPHINT = bool(int(os.environ.get("KERNEL_LOOPHINT", "0")))
            if LOOP > 0:
                _hint = (tuple(mybir.EngineType[e] for e in
                               ("PE", "Activation", "DVE", "SP", "Pool"))
                         if LOOPHINT else ())
                loop_cm = tc.For_i(0, LOOP, 1, hint_engines=_hint)
            else:
                loop_cm = contextlib.nullcontext()
            loop_cm = (loop_cm,)
            Sig = mybir.ActivationFunctionType.Sigmoid

            def mm(out, lhsT, rhs, start, stop):
                # float32r: single-pass fp32 matmul (4x faster than fp32 on
                # the PE, slightly reduced multiply precision on hardware)
                nc.tensor.matmul(out, lhsT=lhsT.bitcast(f32r),
                                 rhs=rhs.bitcast(f32r), start=start, stop=stop)

            def psum_tile(i, shape):
                # Alternate between the two PSUM pools: outside the attention
                # inner loop both pools are idle, giving 4 rotating slots.
                pool = spsum if i % 2 == 0 else opsum
                return pool.tile(shape, f32, tag="s" if i % 2 == 0 else "o",
                                 name="ps")

            with loop_cm[0]:
                bias_t = persist.tile([128, 1], f32, tag="bias", name="bias_t")
                nc.vector.memset(bias_t[:], NEG_LOG_S)
                warm_t = persist.tile([128, 1], f32, tag="warm", name="warm_t")
                nc.scalar.activation(warm_t[:], bias_t[:], Sig, bias=bias_t[:])

                wq_sb = persist.tile([128, 1024], f32r, tag="wq", name="wq_sb")
                wk_sb = persist.tile([128, 1024], f32r, tag="wk", name="wk_sb")
                wv_sb = persist.tile([128, 1024], f32r, tag="wv", name="wv_sb")
                wo_sb = persist.tile([128, 1024], f32r, tag="wo", name="wo_sb")
                xt = [persist.tile([128, 2048], f32r, tag=f"xt{c}", name=f"xt{c}")
                      for c in range(4)]
                nc.sync.dma_start(out=xt[0][:], in_=xt_d[:, 0:2048])
                nc.sync.dma_start(out=wq_sb[:], in_=wq_d[:])
                nc.sync.dma_start(out=wk_sb[:], in_=wk_d[:])
                nc.sync.dma_start(out=wv_sb[:], in_=wv_d[:])
                for c in range(1, 4):
                    nc.sync.dma_start(out=xt[c][:], in_=xt_d[:, 2048 * c:2048 * (c + 1)])
                nc.sync.dma_start(out=wo_sb[:], in_=wo_d[:])

                qt = [persist.tile([128, 2048], f32r, tag=f"qt{m}", name=f"qt{m}")
                      for m in range(2)]
                kt = [persist.tile([128, 2048], f32r, tag=f"kt{m}", name=f"kt{m}")
                      for m in range(2)]
                v = [persist.tile([128, 256], f32r, tag=f"v{t}", name=f"v{t}")
                     for t in range(16)]
                ot = [persist.tile([128, 2048], f32r, tag=f"ot{m}", name=f"ot{m}")
                      for m in range(2)]
                otmp = [persist.tile([64, 512], f32r, tag=f"otmp{m}", name=f"otmp{m}")
                        for m in range(8)]

                # ---- projections (all up front; 4 psum slots available) ----
                pi = 0
                for mc in range(2):
                    for w_sb, dst in ((wq_sb, qt[mc]), (wk_sb, kt[mc])):
                        for qc in range(4):
                            ps = psum_tile(pi, [128, 512]); pi += 1
                            for kc in range(4):
                                mm(ps[:, 0:512],
                                   w_sb[:, 256 * kc + 128 * mc:256 * kc + 128 * mc + 128],
                                   xt[kc][:, 512 * qc:512 * (qc + 1)],
                                   start=(kc == 0), stop=(kc == 3))
                            nc.vector.tensor_copy(dst[:, 512 * qc:512 * (qc + 1)],
                                                  ps[:, 0:512])

                # ---- attention ----
                def p_wave(wave):
                    st = stage.tile([128, 2, 512], f32, tag="pstage", name="pstage")
                    ps = psum_tile(0 if wave % 3 != 2 else 1, [128, 1024])
                    for half in range(2):
                        tck = 2 * wave + half
                        col = slice(512 * half, 512 * (half + 1))
                        for c in range(2):
                            mm(ps[:, col],
                               ot[c][:, 128 * tck:128 * (tck + 1)],
                               wo_sb[:, 512 * c:512 * (c + 1)],
                               start=(c == 0), stop=(c == 1))
                    nc.vector.tensor_copy(st[:, :, :], ps[:, 0:1024])
                    dst = out_d[256 * wave:256 * (wave + 1), :].rearrange(
                        "(t p) m -> p t m", p=128)
                    nc.sync.dma_start(out=dst, in_=st[:])

                def attention(p, jit_v=False, p_waves=False):
                    # head pair p: heads (2p, 2p+1), features [128p,128p+128).
                    # Flat software-pipelined loop over (qc, kc): scores(i+1)
                    # is emitted before sigmoid(i)/attnV(i) so the PE keeps
                    # the ScalarE fed across qc boundaries too.
                    flat = [(qc, kc) for qc in range(4) for kc in range(16)]

                    def emit_v(kc):
                        pv = spsum.tile([128, 256], f32, tag="s", name="psV")
                        for vkc in range(4):
                            mm(pv[:, 0:256],
                               xt[vkc][:, 128 * kc:128 * (kc + 1)],
                               wv_sb[:, 256 * vkc:256 * (vkc + 1)],
                               start=(vkc == 0), stop=(vkc == 3))
                        nc.vector.tensor_copy(v[kc][:], pv[:, 0:256])

                    def emit_scores(qc, kc):
                        ks = slice(128 * kc, 128 * (kc + 1))
                        qs = slice(512 * qc, 512 * (qc + 1))
                        s = spsum.tile([128, 1024], f32, tag="s", name="ps")
                        mm(s[:, 0:512], kt[p][0:64, ks], qt[p][0:64, qs],
                           start=True, stop=True)
                        mm(s[:, 512:1024], kt[p][64:128, ks],
                           qt[p][64:128, qs], start=True, stop=True)
                        return s

                    if jit_v:
                        emit_v(0)
                    psO = None
                    s_next = emit_scores(*flat[0])
                    for i, (qc, kc) in enumerate(flat):
                        s = s_next
                        if i + 1 < len(flat):
                            if jit_v and i + 1 < 16:
                                emit_v(i + 1)
                            s_next = emit_scores(*flat[i + 1])
                        a = apool.tile([128, 1024], f32r, tag="a", name="attn")
                        nc.scalar.activation(a[:], s[:], Sig,
                                             bias=bias_t[:], scale=INV_SQRT_DK)
                        if ABLATE != "attnv":
                            if kc == 0:
                                psO = opsum.tile([64, 1024], f32, tag="o",
                                                 name="psO")
                            mm(psO[0:64, 0:512],
                               v[kc][:, 128 * p:128 * p + 64],
                               a[:, 0:512], start=(kc == 0), stop=(kc == 15))
                            mm(psO[0:64, 512:1024],
                               v[kc][:, 128 * p + 64:128 * p + 128],
                               a[:, 512:1024], start=(kc == 0), stop=(kc == 15))
                            if kc == 15:
                                qs = slice(512 * qc, 512 * (qc + 1))
                                nc.vector.tensor_copy(ot[p][0:64, qs],
                                                      psO[0:64, 0:512])
                                tmp = otmp[4 * p + qc]
                                nc.vector.tensor_copy(tmp[0:64, :],
                                                      psO[0:64, 512:1024])
                                nc.sync.dma_start(out=ot[p][64:128, qs],
                                                  in_=tmp[0:64, :])
                                if p_waves:
                                    p_wave(2 * qc)
                                    p_wave(2 * qc + 1)

                def attention_x(jit_v=False):
                    # single software pipeline across BOTH head pairs
                    flat = [(p, qc, kc) for p in range(2) for qc in range(4)
                            for kc in range(16)]

                    def emit_v(kc):
                        pv = spsum.tile([128, 256], f32, tag="s", name="psV")
                        for vkc in range(4):
                            mm(pv[:, 0:256],
                               xt[vkc][:, 128 * kc:128 * (kc + 1)],
                               wv_sb[:, 256 * vkc:256 * (vkc + 1)],
                               start=(vkc == 0), stop=(vkc == 3))
                        nc.vector.tensor_copy(v[kc][:], pv[:, 0:256])

                    def emit_scores(p, qc, kc):
                        ks = slice(128 * kc, 128 * (kc + 1))
                        qs = slice(512 * qc, 512 * (qc + 1))
                        s = spsum.tile([128, 1024], f32, tag="s", name="ps")
                        mm(s[:, 0:512], kt[p][0:64, ks], qt[p][0:64, qs],
                           start=True, stop=True)
                        mm(s[:, 512:1024], kt[p][64:128, ks],
                           qt[p][64:128, qs], start=True, stop=True)
                        return s

                    if jit_v:
                        emit_v(0)
                    psO = None
                    s_next = emit_scores(*flat[0])
                    for i, (p, qc, kc) in enumerate(flat):
                        s = s_next
                        if i + 1 < len(flat):
                            if jit_v and i + 1 < 16:
                                emit_v(i + 1)
                            s_next = emit_scores(*flat[i + 1])
                        a = apool.tile([128, 1024], f32r, tag="a", name="attn")
                        nc.scalar.activation(a[:], s[:], Sig,
                                             bias=bias_t[:], scale=INV_SQRT_DK)
                        if kc == 0:
                            psO = opsum.tile([64, 1024], f32, tag="o", name="psO")
                        mm(psO[0:64, 0:512], v[kc][:, 128 * p:128 * p + 64],
                           a[:, 0:512], start=(kc == 0), stop=(kc == 15))
                        mm(psO[0:64, 512:1024],
                           v[kc][:, 128 * p + 64:128 * p + 128],
                           a[:, 512:1024], start=(kc == 0), stop=(kc == 15))
                        if kc == 15:
                            qs = slice(512 * qc, 512 * (qc + 1))
                            nc.vector.tensor_copy(ot[p][0:64, qs], psO[0:64, 0:512])
                            tmp = otmp[4 * p + qc]
                            nc.vector.tensor_copy(tmp[0:64, :], psO[0:64, 512:1024])
                            nc.sync.dma_start(out=ot[p][64:128, qs], in_=tmp[0:64, :])

                if ABLATE == "attnv":
                    nc.vector.memset(ot[0][:].bitcast(f32), 0.0)
                    nc.vector.memset(ot[1][:].bitcast(f32), 0.0)
                if not JITV:
                    for tck in range(16):
                        pv = psum_tile(tck, [128, 256])
                        for vkc in range(4):
                            mm(pv[:, 0:256],
                               xt[vkc][:, 128 * tck:128 * (tck + 1)],
                               wv_sb[:, 256 * vkc:256 * (vkc + 1)],
                               start=(vkc == 0), stop=(vkc == 3))
                        nc.vector.tensor_copy(v[tck][:], pv[:, 0:256])
                attention(0, jit_v=JITV)
                attention(1, p_waves=not PTAIL)
                if PTAIL:
                    for wave in range(8):
                        p_wave(wave)

                if DEBUG:
                    for m in range(2):
                        nc.sync.dma_start(out=dbg["qt"][m], in_=qt[m][:])
                        nc.sync.dma_start(out=dbg["kt"][m], in_=kt[m][:])
                    for m in range(2):
                        nc.sync.dma_start(out=dbg["ot"][m], in_=ot[m][:])
                    for t in range(16):
                        nc.sync.dma_start(out=dbg["v"][t], in_=v[t][:])


    nc.compile()
    return nc


def get_nc():
    if "nc" not in _CACHE:
        _CACHE["nc"] = _build_nc()
    return _CACHE["nc"]


def make_in_maps(x, W_q, W_k, W_v, W_o):
    x = np.ascontiguousarray(np.asarray(x, dtype=np.float32))
    ws = [np.asarray(w, dtype=np.float32) for w in (W_q, W_k, W_v, W_o)]
    W_q, W_k, W_v, W_o = ws

    def chunked(a, nchunks):
        # [128*nchunks, m] -> [128, nchunks*m] with chunk-major columns
        m = a.shape[1]
        return np.ascontiguousarray(
            a.reshape(nchunks, 128, m).transpose(1, 0, 2).reshape(128, nchunks * m))

    in_maps = []
    for c in range(8):
        b, g = divmod(c, 2)
        gf = slice(GF * g, GF * (g + 1))
        in_maps.append({
            "xt": chunked(np.ascontiguousarray(x[b].T), 4),
            "wq": chunked(np.ascontiguousarray(W_q[gf, :].T), 4),
            "wk": chunked(np.ascontiguousarray(W_k[gf, :].T), 4),
            "wv": chunked(np.ascontiguousarray(W_v[gf, :].T), 4),
            "wo": chunked(np.ascontiguousarray(W_o[:, gf].T), 2),
        })
    return in_maps


def kernel(x, W_q, W_k, W_v, W_o):
    from concourse.bass_utils import run_bass_kernel_spmd

    nc = get_nc()
    in_maps = make_in_maps(x, W_q, W_k, W_v, W_o)
    res = run_bass_kernel_spmd(nc, in_maps, list(range(8)))
    parts = [res.results[c]["out"] for c in range(8)]
    out = np.stack([parts[2 * b] + parts[2 * b + 1] for b in range(B)])
    return np.ascontiguousarray(out.astype(np.float32))

